# revision 29
# baseline (speedup 1.0000x reference)
"""Trainium2 Bass kernel for MixLinear GEMM (LLM.int8-style dynamic-quant GEMM
with outlier correction).

Math (per the reference):
    xf        = x.astype(f32).reshape(M, K)
    scale_row = max|xf|/127                     (per row)
    q_x       = round(xf / scale_row)           (RNE, values in [-127, 127])
    acc       = q_x @ q_weight.T                (int-valued f32 GEMM)
    y         = acc * scale_row * w_scale + bias + xf[:, ind] @ weight_cache.T

Sharding: M (rows of x) is split 8 ways; weights are replicated. Each core:
  - computes row scales (DVE abs-max reduce + reciprocal),
  - quantizes x in transposed layout (DVE mult + ACT magic-number RNE),
  - main GEMM in fp16 (int values <= 127 are exact in fp16; PE accumulates fp32),
  - outlier + bias via a small fp16 GEMM (33 contraction rows: 32 outlier
    columns of x + a ones row whose weight row is the bias),
  - fused dequant epilogue on DVE.

Host-side prep is layout only: slab slicing, transposes, and exact dtype
widenings (int8 -> f16). All arithmetic happens on device.
"""

import os

import numpy as np

M, K, N = 8192, 4096, 4096
NCORES = 8
ML = M // NCORES  # 1024 rows per core
MT = ML // 128    # 8 m-tiles per core
KT = K // 128     # 32 k-tiles
NBW = 512         # n-block width (one PSUM bank)
NB = N // NBW     # 8 n-blocks
NO = 32           # outlier columns
MAGIC = float(1.5 * 2**23)  # fp32 add/sub forces round-to-nearest-even int

_nc_cache = {}
last_results = None  # BassKernelResults of the most recent run (for profiling)


def _build_nc(ind_cols):
    from contextlib import ExitStack

    import concourse.bacc as bacc
    import concourse.tile as tile
    from concourse import mybir

    f16 = mybir.dt.float16
    f32 = mybir.dt.float32
    Alu = mybir.AluOpType
    Act = mybir.ActivationFunctionType

    from concourse.masks import make_identity

    nc = bacc.Bacc("TRN2", target_bir_lowering=False, debug=False,
                   num_devices=NCORES)
    xT = nc.dram_tensor("xT", [K, ML], f16, kind="ExternalInput").ap()
    qwT = nc.dram_tensor("qwT", [K, N], f16, kind="ExternalInput").ap()
    wcb = nc.dram_tensor("wcb", [NO + 1, N], f16, kind="ExternalInput").ap()
    wscale = nc.dram_tensor("wscale", [1, N], f32, kind="ExternalInput").ap()
    y = nc.dram_tensor("y", [ML, N], f16, kind="ExternalOutput").ap()

    with tile.TileContext(nc) as tc, ExitStack() as ctx:
        singles = ctx.enter_context(tc.tile_pool(name="singles", bufs=1))
        tp = ctx.enter_context(tc.tile_pool(name="tq", bufs=4))
        qwp = ctx.enter_context(tc.tile_pool(name="qw", bufs=2))
        wcbp = ctx.enter_context(tc.tile_pool(name="wcbp", bufs=2))
        wscp = ctx.enter_context(tc.tile_pool(name="wsc", bufs=2))
        epip = ctx.enter_context(tc.tile_pool(name="epi", bufs=3))
        yp = ctx.enter_context(tc.tile_pool(name="yp", bufs=3))

        # xq holds xT on load; quantize overwrites it in place with qxT.
        xq = singles.tile([128, KT, ML], f16)
        pmax = singles.tile([128, ML], f16)
        pmin = singles.tile([128, ML], f16)
        rowmax = singles.tile([128, MT], f32)
        srow = singles.tile([128, MT], f32)
        inv = singles.tile([128, MT], f32)
        invb = singles.tile([128, ML], f32)
        xout = singles.tile([NO + 1, ML], f16)
        oc0 = singles.tile([128, MT, NBW], f32)  # n-block 0 outlier+bias
        ident = singles.tile([128, 128], f16)
        make_identity(nc, ident[:])
        ident32 = singles.tile([128, 128], f32)
        make_identity(nc, ident32[:])
        onesf = singles.tile([128, 128], f32)
        nc.vector.memset(onesf[:], 1.0)

        # --- load x^T (sync ring); outlier rows; qw block 0 (scalar ring) ---
        for g in range(4):
            src = xT[g * 1024:(g + 1) * 1024, :].rearrange("(kt p) m -> p kt m",
                                                           p=128)
            nc.sync.dma_start(out=xq[:, 8 * g:8 * (g + 1), :], in_=src)
        for o, col in enumerate(ind_cols):
            nc.gpsimd.dma_start(out=xout[o:o + 1, :], in_=xT[col:col + 1, :])
        nc.vector.memset(xout[NO:NO + 1, :], 1.0)  # ones row -> bias term

        # --- row scales: scale_row[m] = absmax(x[m, :]) / 127 ---
        # max/min chains over k-tiles (f16, DVE 2x mode; max of f16 values is
        # exact), combined to absmax, then reduced across partitions via PE
        # transpose. (tensor_tensor abs_max is rejected by walrus codegen.)
        nc.vector.tensor_tensor(out=pmax[:], in0=xq[:, 0, :], in1=xq[:, 1, :],
                                op=Alu.max)
        nc.vector.tensor_tensor(out=pmin[:], in0=xq[:, 0, :], in1=xq[:, 1, :],
                                op=Alu.min)
        for kt in range(2, KT):
            nc.vector.tensor_tensor(out=pmax[:], in0=pmax[:], in1=xq[:, kt, :],
                                    op=Alu.max)
            nc.vector.tensor_tensor(out=pmin[:], in0=pmin[:], in1=xq[:, kt, :],
                                    op=Alu.min)
        nc.vector.scalar_tensor_tensor(out=pmax[:], in0=pmin[:], scalar=-1.0,
                                       in1=pmax[:], op0=Alu.mult, op1=Alu.max)
        with tc.tile_pool(name="ptp", bufs=2, space="PSUM") as ptp:
            for c in range(MT):
                pt = ptp.tile([128, 128], f16)
                nc.tensor.transpose(pt[:], pmax[:, c * 128:(c + 1) * 128],
                                    ident[:])
                nc.vector.tensor_reduce(out=rowmax[:, c:c + 1], in_=pt[:],
                                        op=Alu.max, axis=mybir.AxisListType.X)
        nc.vector.tensor_scalar_mul(srow[:], rowmax[:], 1.0 / 127.0)
        nc.vector.reciprocal(inv[:], srow[:])
        # invb[q, t*128+j] = inv[j, t] for all q, via PE with no DRAM hop:
        # diag_t = ident32 * inv[:, t] (per-partition bcast), then
        # ones.T @ diag_t sums the single nonzero per column -> a full
        # partition-broadcast row. All products are 1.0*v or 0.0*v (exact).
        with tc.tile_pool(name="pbc", bufs=2, space="PSUM") as pbc, \
                tc.tile_pool(name="dgp", bufs=2) as dgp:
            for t in range(MT):
                diag_t = dgp.tile([128, 128], f32)
                nc.vector.tensor_tensor(
                    out=diag_t[:], in0=ident32[:],
                    in1=inv[:, t:t + 1].to_broadcast((128, 128)), op=Alu.mult)
                pb_t = pbc.tile([128, 128], f32)
                nc.tensor.matmul(pb_t[:], lhsT=onesf[:], rhs=diag_t[:],
                                 start=True, stop=True)
                nc.scalar.copy(out=invb[:, t * 128:(t + 1) * 128], in_=pb_t[:])


        # --- quantize in place: xq[:, kt, :] = RNE(xT_tile * inv) ---
        for kt in range(KT):
            veng = nc.gpsimd if kt % 4 == 3 else nc.vector
            t_t = tp.tile([128, ML], f32)
            veng.tensor_tensor(out=t_t[:], in0=xq[:, kt, :], in1=invb[:],
                               op=Alu.mult)
            veng.tensor_scalar_add(t_t[:], t_t[:], MAGIC)
            nc.scalar.activation(out=xq[:, kt, :], in_=t_t[:], func=Act.Copy,
                                 bias=-MAGIC)

        psA = ctx.enter_context(tc.tile_pool(name="psA", bufs=8, space="PSUM"))
        psB = psA

        def load_nb(nb, eng=None):
            eng = eng or nc.scalar
            ns = nb * NBW
            qw_t = qwp.tile([128, KT, NBW], f16)
            for g in range(4):
                src = qwT[g * 1024:(g + 1) * 1024, ns:ns + NBW].rearrange(
                    "(kt p) n -> p kt n", p=128)
                eng.dma_start(out=qw_t[:, g * 8:(g + 1) * 8, :], in_=src)
            wcb_t = wcbp.tile([NO + 1, NBW], f16)
            eng.dma_start(out=wcb_t[:], in_=wcb[:, ns:ns + NBW])
            wsc_t = wscp.tile([128, NBW], f32)
            nc.gpsimd.dma_start(out=wsc_t[:],
                                in_=wscale[:, ns:ns + NBW].to_broadcast((128, NBW)))
            return qw_t, wcb_t, wsc_t

        def epilogue(nb, mt, pA, oc):
            ns = nb * NBW
            te = epip.tile([128, NBW], f32)
            nc.vector.scalar_tensor_tensor(
                out=te[:], in0=pA[:], scalar=srow[:, mt:mt + 1], in1=wsc_ts[nb][:],
                op0=Alu.mult, op1=Alu.mult)
            y_t = yp.tile([128, NBW], f16)
            nc.vector.tensor_tensor(out=y_t[:], in0=te[:], in1=oc[:], op=Alu.add)
            nc.sync.dma_start(out=y[mt * 128:(mt + 1) * 128, ns:ns + NBW],
                              in_=y_t[:])

        wsc_ts = {}

        # --- n-block 0: outliers up front, then k-outer so the PE consumes
        # quantized tiles as they are produced (7 concurrent PSUM groups).
        qw_t, wcb_t, wsc_t = load_nb(0, eng=nc.gpsimd)
        wsc_ts[0] = wsc_t
        for mt in range(MT):
            pB = psB.tile([128, NBW], f32, tag="pA")
            nc.tensor.matmul(pB[:], lhsT=xout[:, mt * 128:(mt + 1) * 128],
                             rhs=wcb_t[:], start=True, stop=True)
            nc.scalar.copy(out=oc0[:, mt, :], in_=pB[:])
        pAs = [psA.tile([128, NBW], f32, name=f"pA{i}", tag="pA")
               for i in range(MT)]
        for kt in range(KT):
            for mt in range(MT):
                nc.tensor.matmul(pAs[mt][:], lhsT=xq[:, kt, mt * 128:(mt + 1) * 128],
                                 rhs=qw_t[:, kt, :],
                                 start=(kt == 0), stop=(kt == KT - 1))
        for mt in range(MT):
            epilogue(0, mt, pAs[mt], oc0[:, mt, :])

        # --- n-blocks 1..7: m-outer streaming ---
        for nb in range(1, NB):
            qw_t, wcb_t, wsc_t = load_nb(nb)
            wsc_ts[nb] = wsc_t
            for mt in range(MT):
                ms = mt * 128
                pA = psA.tile([128, NBW], f32, tag="pA")
                for kt in range(KT):
                    nc.tensor.matmul(pA[:], lhsT=xq[:, kt, ms:ms + 128],
                                     rhs=qw_t[:, kt, :],
                                     start=(kt == 0), stop=(kt == KT - 1))
                pB = psB.tile([128, NBW], f32, tag="pA")
                nc.tensor.matmul(pB[:], lhsT=xout[:, ms:ms + 128], rhs=wcb_t[:],
                                 start=True, stop=True)
                epilogue(nb, mt, pA, pB)

    nc.compile()
    return nc


def kernel(x, q_weight, scale_col, weight_cache, ind, **_unused):
    global last_results
    from concourse.bass_utils import run_bass_kernel_spmd

    out_dtype = x.dtype  # float16
    xf = np.asarray(x).reshape(M, K)
    qwT_np = np.ascontiguousarray(np.asarray(q_weight).T).astype(np.float16)
    wcb_np = np.ascontiguousarray(np.concatenate(
        [np.asarray(weight_cache).T.astype(np.float16),
         np.asarray(scale_col)[:, 1].astype(np.float16)[None, :]], axis=0))
    wscale_np = np.ascontiguousarray(
        np.asarray(scale_col)[:, 0].astype(np.float32)[None, :])
    ind_cols = tuple(int(i) for i in np.asarray(ind))

    nc = _nc_cache.get(ind_cols)
    if nc is None:
        nc = _build_nc(ind_cols)
        _nc_cache[ind_cols] = nc

    in_maps = []
    for c in range(NCORES):
        slab = xf[c * ML:(c + 1) * ML]
        in_maps.append({
            "xT": np.ascontiguousarray(slab.T),
            "qwT": qwT_np,
            "wcb": wcb_np,
            "wscale": wscale_np,
        })

    res = run_bass_kernel_spmd(nc, in_maps, core_ids=list(range(NCORES)))
    last_results = res
    out = np.concatenate([res.results[c]["y"] for c in range(NCORES)], axis=0)
    return out.reshape(np.asarray(x).shape).astype(out_dtype, copy=False)


# revision 30
# speedup vs baseline: 1.2338x; 1.2338x over previous
"""Trainium2 Bass kernel for MixLinear GEMM (LLM.int8-style dynamic-quant GEMM
with outlier correction).

Math (per the reference):
    xf        = x.astype(f32).reshape(M, K)
    scale_row = max|xf|/127                     (per row)
    q_x       = round(xf / scale_row)           (RNE, values in [-127, 127])
    acc       = q_x @ q_weight.T                (int-valued f32 GEMM)
    y         = acc * scale_row * w_scale + bias + xf[:, ind] @ weight_cache.T

Sharding: M (rows of x) is split 8 ways; weights are replicated. Each core:
  - computes row scales (DVE abs-max reduce + reciprocal),
  - quantizes x in transposed layout (DVE mult + ACT magic-number RNE),
  - main GEMM in fp16 (int values <= 127 are exact in fp16; PE accumulates fp32),
  - outlier + bias via a small fp16 GEMM (33 contraction rows: 32 outlier
    columns of x + a ones row whose weight row is the bias),
  - fused dequant epilogue on DVE.

Host-side prep is layout only: slab slicing, transposes, and exact dtype
widenings (int8 -> f16). All arithmetic happens on device.
"""

import os

import numpy as np

M, K, N = 8192, 4096, 4096
NCORES = 8
ML = M // NCORES  # 1024 rows per core
MT = ML // 128    # 8 m-tiles per core
KT = K // 128     # 32 k-tiles
NBW = 512         # n-block width (one PSUM bank)
NB = N // NBW     # 8 n-blocks
NO = 32           # outlier columns
MAGIC = float(1.5 * 2**23)  # fp32 add/sub forces round-to-nearest-even int

_nc_cache = {}
last_results = None  # BassKernelResults of the most recent run (for profiling)


def _build_nc(ind_cols):
    from contextlib import ExitStack

    import concourse.bacc as bacc
    import concourse.tile as tile
    from concourse import mybir

    f16 = mybir.dt.float16
    f32 = mybir.dt.float32
    Alu = mybir.AluOpType
    Act = mybir.ActivationFunctionType

    from concourse.masks import make_identity

    nc = bacc.Bacc("TRN2", target_bir_lowering=False, debug=False,
                   num_devices=NCORES)
    xT = nc.dram_tensor("xT", [K, ML], f16, kind="ExternalInput").ap()
    qwT = nc.dram_tensor("qwT", [K, N], f16, kind="ExternalInput").ap()
    wcb = nc.dram_tensor("wcb", [NO + 1, N], f16, kind="ExternalInput").ap()
    wscale = nc.dram_tensor("wscale", [1, N], f32, kind="ExternalInput").ap()
    y = nc.dram_tensor("y", [ML, N], f16, kind="ExternalOutput").ap()

    with tile.TileContext(nc) as tc, ExitStack() as ctx:
        singles = ctx.enter_context(tc.tile_pool(name="singles", bufs=1))
        tp = ctx.enter_context(tc.tile_pool(name="tq", bufs=4))
        qwp = ctx.enter_context(tc.tile_pool(name="qw", bufs=2))
        wcbp = ctx.enter_context(tc.tile_pool(name="wcbp", bufs=2))
        wscp = ctx.enter_context(tc.tile_pool(name="wsc", bufs=2))
        epip = ctx.enter_context(tc.tile_pool(name="epi", bufs=3))
        yp = ctx.enter_context(tc.tile_pool(name="yp", bufs=3))

        # xq holds xT on load; quantize overwrites it in place with qxT.
        xq = singles.tile([128, KT, ML], f16)
        pmax = singles.tile([128, ML], f16)
        pmin = singles.tile([128, ML], f16)
        rowmax = singles.tile([128, MT], f32)
        srow = singles.tile([128, MT], f32)
        inv = singles.tile([128, MT], f32)
        invb = singles.tile([128, ML], f32)
        xout = singles.tile([NO + 1, ML], f16)
        oc0 = singles.tile([128, MT, NBW], f32)  # n-block 0 outlier+bias
        ident = singles.tile([128, 128], f16)
        make_identity(nc, ident[:])
        ident32 = singles.tile([128, 128], f32)
        make_identity(nc, ident32[:])
        onesf = singles.tile([128, 128], f32)
        nc.vector.memset(onesf[:], 1.0)

        # --- load x^T (sync ring); outlier rows; qw block 0 (scalar ring) ---
        for g in range(4):
            src = xT[g * 1024:(g + 1) * 1024, :].rearrange("(kt p) m -> p kt m",
                                                           p=128)
            nc.sync.dma_start(out=xq[:, 8 * g:8 * (g + 1), :], in_=src)
        for o, col in enumerate(ind_cols):
            nc.gpsimd.dma_start(out=xout[o:o + 1, :], in_=xT[col:col + 1, :])
        nc.vector.memset(xout[NO:NO + 1, :], 1.0)  # ones row -> bias term

        # --- row scales: scale_row[m] = absmax(x[m, :]) / 127 ---
        # max/min chains over k-tiles (f16, DVE 2x mode; max of f16 values is
        # exact), combined to absmax, then reduced across partitions via PE
        # transpose. (tensor_tensor abs_max is rejected by walrus codegen.)
        nc.vector.tensor_tensor(out=pmax[:], in0=xq[:, 0, :], in1=xq[:, 1, :],
                                op=Alu.max)
        nc.vector.tensor_tensor(out=pmin[:], in0=xq[:, 0, :], in1=xq[:, 1, :],
                                op=Alu.min)
        for kt in range(2, KT):
            nc.vector.tensor_tensor(out=pmax[:], in0=pmax[:], in1=xq[:, kt, :],
                                    op=Alu.max)
            nc.vector.tensor_tensor(out=pmin[:], in0=pmin[:], in1=xq[:, kt, :],
                                    op=Alu.min)
        nc.vector.scalar_tensor_tensor(out=pmax[:], in0=pmin[:], scalar=-1.0,
                                       in1=pmax[:], op0=Alu.mult, op1=Alu.max)
        with tc.tile_pool(name="ptp", bufs=2, space="PSUM") as ptp:
            for c in range(MT):
                pt = ptp.tile([128, 128], f16)
                nc.tensor.transpose(pt[:], pmax[:, c * 128:(c + 1) * 128],
                                    ident[:])
                nc.vector.tensor_reduce(out=rowmax[:, c:c + 1], in_=pt[:],
                                        op=Alu.max, axis=mybir.AxisListType.X)
        nc.vector.tensor_scalar_mul(srow[:], rowmax[:], 1.0 / 127.0)
        nc.vector.reciprocal(inv[:], srow[:])
        # invb[q, t*128+j] = inv[j, t] for all q, via PE with no DRAM hop:
        # diag_t = ident32 * inv[:, t] (per-partition bcast), then
        # ones.T @ diag_t sums the single nonzero per column -> a full
        # partition-broadcast row. All products are 1.0*v or 0.0*v (exact).
        with tc.tile_pool(name="pbc", bufs=2, space="PSUM") as pbc, \
                tc.tile_pool(name="dgp", bufs=2) as dgp:
            for t in range(MT):
                diag_t = dgp.tile([128, 128], f32)
                nc.vector.tensor_tensor(
                    out=diag_t[:], in0=ident32[:],
                    in1=inv[:, t:t + 1].to_broadcast((128, 128)), op=Alu.mult)
                pb_t = pbc.tile([128, 128], f32)
                nc.tensor.matmul(pb_t[:], lhsT=onesf[:], rhs=diag_t[:],
                                 start=True, stop=True)
                nc.scalar.copy(out=invb[:, t * 128:(t + 1) * 128], in_=pb_t[:])


        # --- quantize in place: xq[:, kt, :] = RNE(xT_tile * inv) ---
        for kt in range(KT):
            t_t = tp.tile([128, ML], f32)
            nc.vector.tensor_tensor(out=t_t[:], in0=xq[:, kt, :], in1=invb[:],
                                    op=Alu.mult)
            nc.vector.tensor_scalar_add(t_t[:], t_t[:], MAGIC)
            nc.scalar.activation(out=xq[:, kt, :], in_=t_t[:], func=Act.Copy,
                                 bias=-MAGIC)

        psA = ctx.enter_context(tc.tile_pool(name="psA", bufs=8, space="PSUM"))
        psB = psA

        def load_nb(nb, eng=None):
            eng = eng or nc.scalar
            ns = nb * NBW
            qw_t = qwp.tile([128, KT, NBW], f16)
            for g in range(4):
                src = qwT[g * 1024:(g + 1) * 1024, ns:ns + NBW].rearrange(
                    "(kt p) n -> p kt n", p=128)
                eng.dma_start(out=qw_t[:, g * 8:(g + 1) * 8, :], in_=src)
            wcb_t = wcbp.tile([NO + 1, NBW], f16)
            eng.dma_start(out=wcb_t[:], in_=wcb[:, ns:ns + NBW])
            wsc_t = wscp.tile([128, NBW], f32)
            nc.gpsimd.dma_start(out=wsc_t[:],
                                in_=wscale[:, ns:ns + NBW].to_broadcast((128, NBW)))
            return qw_t, wcb_t, wsc_t

        def epilogue(nb, mt, pA, oc):
            ns = nb * NBW
            te = epip.tile([128, NBW], f32)
            nc.vector.scalar_tensor_tensor(
                out=te[:], in0=pA[:], scalar=srow[:, mt:mt + 1], in1=wsc_ts[nb][:],
                op0=Alu.mult, op1=Alu.mult)
            y_t = yp.tile([128, NBW], f16)
            nc.vector.tensor_tensor(out=y_t[:], in0=te[:], in1=oc[:], op=Alu.add)
            nc.sync.dma_start(out=y[mt * 128:(mt + 1) * 128, ns:ns + NBW],
                              in_=y_t[:])

        wsc_ts = {}

        # --- n-block 0: outliers up front, then k-outer so the PE consumes
        # quantized tiles as they are produced (7 concurrent PSUM groups).
        qw_t, wcb_t, wsc_t = load_nb(0, eng=nc.sync)
        wsc_ts[0] = wsc_t
        for mt in range(MT):
            pB = psB.tile([128, NBW], f32, tag="pA")
            nc.tensor.matmul(pB[:], lhsT=xout[:, mt * 128:(mt + 1) * 128],
                             rhs=wcb_t[:], start=True, stop=True)
            nc.scalar.copy(out=oc0[:, mt, :], in_=pB[:])
        pAs = [psA.tile([128, NBW], f32, name=f"pA{i}", tag="pA")
               for i in range(MT)]
        for kt in range(KT):
            for mt in range(MT):
                nc.tensor.matmul(pAs[mt][:], lhsT=xq[:, kt, mt * 128:(mt + 1) * 128],
                                 rhs=qw_t[:, kt, :],
                                 start=(kt == 0), stop=(kt == KT - 1))
        for mt in range(MT):
            epilogue(0, mt, pAs[mt], oc0[:, mt, :])

        # --- n-blocks 1..7: m-outer streaming ---
        for nb in range(1, NB):
            qw_t, wcb_t, wsc_t = load_nb(nb)
            wsc_ts[nb] = wsc_t
            for mt in range(MT):
                ms = mt * 128
                pA = psA.tile([128, NBW], f32, tag="pA")
                for kt in range(KT):
                    nc.tensor.matmul(pA[:], lhsT=xq[:, kt, ms:ms + 128],
                                     rhs=qw_t[:, kt, :],
                                     start=(kt == 0), stop=(kt == KT - 1))
                pB = psB.tile([128, NBW], f32, tag="pA")
                nc.tensor.matmul(pB[:], lhsT=xout[:, ms:ms + 128], rhs=wcb_t[:],
                                 start=True, stop=True)
                epilogue(nb, mt, pA, pB)

    nc.compile()
    return nc


def kernel(x, q_weight, scale_col, weight_cache, ind, **_unused):
    global last_results
    from concourse.bass_utils import run_bass_kernel_spmd

    out_dtype = x.dtype  # float16
    xf = np.asarray(x).reshape(M, K)
    qwT_np = np.ascontiguousarray(np.asarray(q_weight).T).astype(np.float16)
    wcb_np = np.ascontiguousarray(np.concatenate(
        [np.asarray(weight_cache).T.astype(np.float16),
         np.asarray(scale_col)[:, 1].astype(np.float16)[None, :]], axis=0))
    wscale_np = np.ascontiguousarray(
        np.asarray(scale_col)[:, 0].astype(np.float32)[None, :])
    ind_cols = tuple(int(i) for i in np.asarray(ind))

    nc = _nc_cache.get(ind_cols)
    if nc is None:
        nc = _build_nc(ind_cols)
        _nc_cache[ind_cols] = nc

    in_maps = []
    for c in range(NCORES):
        slab = xf[c * ML:(c + 1) * ML]
        in_maps.append({
            "xT": np.ascontiguousarray(slab.T),
            "qwT": qwT_np,
            "wcb": wcb_np,
            "wscale": wscale_np,
        })

    res = run_bass_kernel_spmd(nc, in_maps, core_ids=list(range(NCORES)))
    last_results = res
    out = np.concatenate([res.results[c]["y"] for c in range(NCORES)], axis=0)
    return out.reshape(np.asarray(x).shape).astype(out_dtype, copy=False)
